# revision 77
# baseline (speedup 1.0000x reference)
"""Trainium2 Bass kernel for nn_Attention_54150947668207 (sparse channel attention).

Algorithm restructure (verified exact vs reference in fp32, rel 3.4e-7):
  - dwconv3x3 per channel on q,k,v (depthwise, SAME pad)
  - per (batch,head): attn = normalize(q) @ normalize(k)^T over pixels; the 4
    top-k masked softmaxes combine into ONE matrix A_comb = sum_i w_i*softmax_i
    (top-k via rank-count, col-scaled before ranking; exp needs no max-sub)
  - M_combT = blockdiag(A_comb)^T @ Wproj^T folds all four attn@v matmuls AND
    the 1x1 projection into ONE [384,384]@[384,px] matmul per pixel shard.

Sharding (8 cores, SPMD):
  - attention phase: core i handles batch i//4, heads {2*(i%4), 2*(i%4)+1}
  - projection phase: core i handles batch i//4, image rows 32*(i%4)..+32
  - connected by one AllGather of M_combT ([96,384] f16) in 4-core groups.

Implementation notes (measured on HW):
  - fp16 on device (inputs host-cast); matmuls accumulate fp32 in PSUM.
  - dwconv: 6 taps as PE diag-matmuls (fp32 PSUM acc), 3 taps on DVE as
    tensor_scalar(4x) products + tensor_tensor(2x) adds in half-chunks.
    Diag lhsT padded to 128 cols -> Fast Weight Load; psum is full-height
    with junk rows above npart, never drained.
  - attn Gram lhsT reads 128 cols from the transposed tiles (own group's 96
    + 32 overrun, junk lands in unread psum rows 96-127) -> FWL: attn MMs
    issue at ~51ns instead of ~215ns. Transpose dst stays contiguous (the
    strided-dst form knocks DMA_TRANSPOSE off its fast path, ~2x slower).
  - attn matmuls for chunk i are emitted after chunk i+1's conv taps, so the
    Tensor FIFO never head-of-line blocks on chunk i's DVE tail/transpose.
  - DMA_TRANSPOSE occupies its issuing queue for the whole ~4us transfer
    plus its semaphore wait: transposes ride the sync queue, where they only
    delay loads already prefetched ahead (on ACT they stall PSUM drains,
    which backpressures the PE mid-chunk).
  - chain scratch rings: DVE-tap intermediates rotate in ring A/X, finals in
    ring B, so next chunk's PSUM drains never WAR against this chunk's
    transpose/Square readers (the v1 layout re-throttled HAM every chunk).
  - v-conv moved after the attention phase: group 0 runs ALL 9 taps on PE
    (keeps the DVE FIFO clear so the post-attn chain starts at attn3),
    groups 1/2 (6 PE + 3 DVE taps) cover the collective window.
  - collective: AllGather of the tiny fp16 A_comb blocks (9.2KB in) instead
    of M_combT (74KB) — the Wproj multiply runs post-gather as 8 small MMs
    (~25us -> ~14us transfer+skew; trigger latency is a fixed ~11.5us).
  - 4 masked softmaxes vectorized over the branch axis (6 DVE ops total);
    thresholds ride in via the ksthr input.
  - walrus here accepts ONE sync wait per instruction: _split_multi_waits
    rewrites Tile's multi-wait instructions into NoOp-carried single waits.
  - the LAST q/k chunk runs all 9 taps on PE (fp32 PSUM): removes the serial
    DVE tap chain from the critical path into the tail and cuts rounding on
    the rank-sensitive attention inputs.
HW exec ~237us/8 cores (neuron-profile), rel err 9.1e-3 vs fp32 reference.
"""
import sys

for _p in ("/opt/trn_rl_repo",):
    if _p not in sys.path:
        sys.path.insert(0, _p)

import numpy as np
from contextlib import ExitStack

import concourse.bass as bass
import concourse.tile as tile
from concourse import mybir
from concourse.bass_utils import run_bass_kernel_spmd

F32 = mybir.dt.float32
F16 = mybir.dt.float16
AOT = mybir.AluOpType
ACTF = mybir.ActivationFunctionType

C = 384
HEADS = 8
CH = 48          # channels per head
H = W = 128
HW = H * W
B = 2
C2 = 96          # channels per core in attention phase (2 heads)
KS = (CH // 2, CH * 2 // 3, CH * 3 // 4, CH * 4 // 5)  # 24, 32, 36, 38

# tap order t = 3*ky + kx, offsets (dy,dx) = (ky-1, kx-1)
N_TAPS = 9
CFG = {
    "pe_taps_qk": (0, 1, 3, 4, 5, 7),   # taps done on PE (fp32 psum acc)
    "pe_taps_v": (0, 1, 3, 4, 5, 7),
    "big_xpose": True,               # one dma_start_transpose per chunk
    "dve_style": "ts_tt",            # "stt" | "ts_tt" | "act_tt"
    "split_waits": True,
    "max_waits": 1,
}

NCHUNK = 4            # q/k processed in 4 chunks of 32 rows
ROWS_PER_CHUNK = 32
CHUNK_PX = ROWS_PER_CHUNK * W   # 4096
SUB = 512             # psum sub-chunk width for PE dwconv
VROWS = 32            # v shard rows per core
VPX = VROWS * W       # 4096


def _split_multi_waits(nc, max_waits=1):
    """walrus in this container accepts limited sync waits per instruction;
    split extras into preceding single-wait NoOps on the same engine."""
    n = 0
    for f in nc.m.functions:
        for blk in f.blocks:
            new_insts = []
            for inst in blk.instructions:
                si = getattr(inst, "sync_info", None)
                if si is not None and si.on_wait and len(si.on_wait) > max_waits:
                    waits = list(si.on_wait)
                    for wcond in waits[:-max_waits]:
                        nop = mybir.InstNoOp(
                            name=f"I-waitsplit-{nc.next_id()}",
                            ins=[], outs=[],
                            engine=inst.engine,
                            sync_info=mybir.SyncInfo(on_wait=[wcond], on_update=[]),
                        )
                        new_insts.append(nop)
                        n += 1
                    si.on_wait = waits[-max_waits:]
                new_insts.append(inst)
            blk.instructions = new_insts
    return n


def _emit_dwconv(nc, pool, psum_dw, xpad, w9, diags, dwp, out_tag,
                 npart, pe_taps, nrows, out_final=None, defer_dve=False):
    """Depthwise 3x3 over nrows output rows.

    PE taps accumulate in fp32 PSUM (diag matmuls); remaining taps run as
    DVE tensor_scalar(4x) products + tensor_tensor(2x) adds.
    Returns the final output AP ([npart, nrows*W] f16), or with
    defer_dve=True a (pe_result_ap, emit_dve_fn) pair so the DVE tap chain
    can be queued later (keeps the DVE FIFO free for critical-path work).
    """
    dve_taps = [t for t in range(N_TAPS) if t not in pe_taps]
    npx = nrows * W
    nsub = npx // 1024
    rows_per_sub = 1024 // W  # 8

    def shifted(t, r_lo, nr):
        ky, kx = divmod(t, 3)
        return xpad[:, r_lo + ky:r_lo + ky + nr, kx:kx + W]

    _ctr = [0]

    def alloc(tag):
        _ctr[0] += 1
        t = dwp.tile([npart, npx], F16, tag=tag, name=f"{out_tag}_{tag}{_ctr[0]}")
        return t[:]

    n_dve = len(dve_taps)
    assert pe_taps
    cur = out_final if (n_dve == 0 and out_final is not None) else alloc(out_tag + "A")
    oc3 = cur.rearrange("p (r w) -> p r w", w=W)
    for s in range(nsub):
        r_lo = s * rows_per_sub
        # full-height psum: diag lhsT is padded to 128 cols (enables FWL);
        # rows npart..127 are junk and never drained
        ps = psum_dw.tile([128, 1024], F32, tag="psdw")
        for half in range(2):
            for i, t in enumerate(pe_taps):
                nc.tensor.matmul(
                    ps[:, half * 512:half * 512 + 512], diags[t],
                    shifted(t, r_lo + half * 4, 4),
                    start=(i == 0), stop=(i == len(pe_taps) - 1))
        nc.scalar.copy(oc3[:, r_lo:r_lo + rows_per_sub, :], ps[0:npart, :])

    def emit_dve(cur=cur, oc3=oc3):
        # product tree: all w_t*shift_t products and their pairwise sums
        # depend only on the padded input, so they run during the chunk's
        # PE taps; exactly ONE tensor_tensor per half depends on the PSUM
        # drain — minimizing the transpose's wait on this chain.
        if n_dve == 0:
            return cur
        fin = out_final if out_final is not None else alloc(out_tag + "B")
        fi3 = fin.rearrange("p (r w) -> p r w", w=W)
        hr = nrows // 2
        for hh in range(2):
            r0h = hh * hr
            prev = oc3[:, r0h:r0h + hr, :]
            for j, t in enumerate(dve_taps):
                tmp = dwp.tile([npart, hr * W], F16, tag="dwtmp",
                               name=f"{out_tag}_tmp{_ctr[0]}_{hh}")
                _ctr[0] += 1
                tm3 = tmp[:].rearrange("p (r w) -> p r w", w=W)
                nc.vector.tensor_scalar(
                    tm3, shifted(t, r0h, hr), w9[:, t:t + 1], None, AOT.mult)
                if j == n_dve - 1:
                    dst = fi3[:, r0h:r0h + hr, :]
                else:
                    nxt = dwp.tile([npart, hr * W], F16, tag=out_tag + "X",
                                   name=f"{out_tag}_acc{_ctr[0]}_{hh}")
                    _ctr[0] += 1
                    dst = nxt[:].rearrange("p (r w) -> p r w", w=W)
                nc.vector.tensor_tensor(dst, tm3, prev, AOT.add)
                prev = dst
        return fin

    if defer_dve:
        return cur, emit_dve
    return emit_dve()


def build_kernel():
    nc = bass.Bass("TRN2", target_bir_lowering=False, debug=False, num_devices=8)

    # ---- DRAM I/O ----
    qs = nc.declare_dram_parameter("qs", [C2, 130, 130], F16, isOutput=False)
    ks = nc.declare_dram_parameter("ks", [C2, 130, 130], F16, isOutput=False)
    vs = nc.declare_dram_parameter("vs", [C, 34, 130], F16, isOutput=False)
    wq9 = nc.declare_dram_parameter("wq9", [C2, 9], F32, isOutput=False)
    wk9 = nc.declare_dram_parameter("wk9", [C2, 9], F32, isOutput=False)
    wv9 = nc.declare_dram_parameter("wv9", [C, 9], F32, isOutput=False)
    dgqk = nc.declare_dram_parameter("dgqk", [2, 9, C2, 128], F16, isOutput=False)
    dgv = nc.declare_dram_parameter("dgv", [9, 3, 128, 128], F16, isOutput=False)
    wpT = nc.declare_dram_parameter("wpT", [CH, HEADS, C], F16, isOutput=False)
    tempv = nc.declare_dram_parameter("tempv", [C2, 1], F32, isOutput=False)
    attwv = nc.declare_dram_parameter("attwv", [C2, 4], F32, isOutput=False)
    ksthr = nc.declare_dram_parameter("ksthr", [C2, 4], F32, isOutput=False)
    out_ext = nc.declare_dram_parameter("out", [3, 128, VPX], F16, isOutput=True)

    with tile.TileContext(nc) as tc, ExitStack() as ctx:
        pool = ctx.enter_context(tc.tile_pool(name="sbuf", bufs=1))
        pads = ctx.enter_context(tc.tile_pool(name="pads", bufs=3))
        vpads = ctx.enter_context(tc.tile_pool(name="vpads", bufs=3))
        xpT = ctx.enter_context(tc.tile_pool(name="xpT", bufs=2))
        dwp = ctx.enter_context(tc.tile_pool(name="dwp", bufs=2))
        psum_dw = ctx.enter_context(tc.tile_pool(name="psdw", bufs=2, space="PSUM"))
        psum_a = ctx.enter_context(tc.tile_pool(name="psa", bufs=1, space="PSUM"))
        psum_o = ctx.enter_context(tc.tile_pool(name="pso", bufs=3, space="PSUM"))
        obuf = ctx.enter_context(tc.tile_pool(name="obuf", bufs=3))
        dram = ctx.enter_context(tc.tile_pool(name="dram", bufs=1, space="DRAM"))

        # ---- constants ----
        # dgqk on the sync queue (needed by the first conv matmuls); all
        # other constants ride the scalar/gpsimd queues so chunk-0/1 input
        # loads start immediately after dgqk.
        dgqk_t = pool.tile([C2, 2, 9, 128], F16, tag="dgqk")
        nc.sync.dma_start(dgqk_t[:], dgqk.ap().rearrange("a t c e -> c a t e"))
        w9q = pool.tile([C2, 9], F32); nc.scalar.dma_start(w9q[:], wq9.ap())
        w9k = pool.tile([C2, 9], F32); nc.scalar.dma_start(w9k[:], wk9.ap())
        w9v = pool.tile([128, 3, 9], F32)
        for ct in range(3):
            nc.scalar.dma_start(w9v[:, ct, :], wv9.ap()[128 * ct:128 * (ct + 1), :])
        dgv_t = pool.tile([128, 9, 3, 128], F16, tag="dgvt")
        nc.gpsimd.dma_start(dgv_t[:], dgv.ap().rearrange("t g c e -> c t g e"))
        diag_q = {t: dgqk_t[:, 0, t, :] for t in range(9)}
        diag_k = {t: dgqk_t[:, 1, t, :] for t in range(9)}
        diag_v = {(t, ct): dgv_t[:, t, ct, :]
                  for t in range(9) for ct in range(3)}
        wpa = pool.tile([CH, HEADS, C], F16, tag="wpa")
        nc.scalar.dma_start(wpa[:], wpT.ap())
        tmpv = pool.tile([C2, 1], F32); nc.scalar.dma_start(tmpv[:], tempv.ap())
        attw = pool.tile([C2, 4], F32); nc.scalar.dma_start(attw[:], attwv.ap())
        ksthr_t = pool.tile([C2, 4], F32); nc.scalar.dma_start(ksthr_t[:], ksthr.ap())

        # ---- q/k dwconv + transpose, interleaved by chunk ----
        # attn MMs for chunk ci are emitted after chunk ci+1's conv taps so
        # the PE FIFO never head-of-line blocks on chunk ci's DVE tail +
        # transpose. v-conv runs AFTER the attention phase: its PE taps cover
        # the post-attention serial chain and the AllGather window.
        sumsq = {}
        vdw = pool.tile([128, 3, VPX], F16, tag="vdw")
        xT = {}   # (name, ci) -> per-chunk transposed tile [128, 32, 128]
        ps_attn = psum_a.tile([128, C2], F32, tag="psattn")
        # v pads: 2-slot ring; vp2 loads lazily into vp0's slot
        vps = {}

        def load_vp(ct):
            # gpsimd queue: keeps the sync DMA queue free for the
            # latency-critical post-attn round trips
            vp = vpads.tile([128, 34, 130], F16, tag="vpad", name=f"vp{ct}")
            nc.gpsimd.dma_start(vp[:], vs.ap()[128 * ct:128 * (ct + 1), :, :])
            vps[ct] = vp

        def emit_attn(ci):
            # lhsT reads 128 columns (own group's 96 + 32 overrun into the
            # next): enables Fast Weight Load (needs NumWeights==128); the
            # overrun cols only write psum rows 96-127, which are unread.
            qTt, kTt = xT[("q", ci)], xT[("k", ci)]
            for jj in range(32):
                j = 32 * ci + jj
                nc.tensor.matmul(ps_attn[:], qTt[:, C2 * jj:C2 * jj + 128],
                                 kTt[:, C2 * jj:C2 * jj + C2],
                                 start=(j == 0), stop=(j == 127))

        def emit_vconv(ct, taps=None):
            taps = taps if taps is not None else CFG["pe_taps_v"]
            diags_v = {t: diag_v[(t, ct)] for t in taps}
            return _emit_dwconv(nc, pool, psum_dw, vps[ct], w9v[:, ct, :],
                                diags_v, dwp, "vw", 128, taps, VROWS,
                                out_final=vdw[:, ct, :])

        for ci in range(NCHUNK):
            r0 = ci * ROWS_PER_CHUNK
            xq = pads.tile([C2, 34, 130], F16, tag="pad", name=f"xq{ci}")
            nc.sync.dma_start(xq[:], qs.ap()[:, r0:r0 + 34, :])
            xk = pads.tile([C2, 34, 130], F16, tag="pad", name=f"xk{ci}")
            nc.sync.dma_start(xk[:], ks.ap()[:, r0:r0 + 34, :])
            dws = {}
            for name, xpad, w9, diags in (("q", xq, diag_q, None), ("k", xk, diag_k, None)):
                w9_ = w9q if name == "q" else w9k
                dg_ = diag_q if name == "q" else diag_k
                taps = CFG["pe_taps_qk"] if ci < NCHUNK - 1 else tuple(range(9))
                dw = _emit_dwconv(nc, pool, psum_dw, xpad, w9_, dg_,
                                  dwp, "dw", C2, taps, ROWS_PER_CHUNK)
                dws[name] = dw
                for hh in range(2):
                    sq = dwp.tile([C2, CHUNK_PX // 2], F16, tag="dwtmp",
                                  name=f"sq_{name}{ci}_{hh}")
                    ss = pool.tile([C2, 1], F32, tag=f"ss_{name}{ci}_{hh}")
                    nc.scalar.activation(
                        sq[:], dw[:, hh * (CHUNK_PX // 2):(hh + 1) * (CHUNK_PX // 2)],
                        ACTF.Square, accum_out=ss[:])
                    sumsq[(name, ci, hh)] = ss
            # sync queue: DMA_TRANSPOSE occupies its issuing queue for the
            # whole ~4us transfer plus its wait on the DVE chain; on sync it
            # only delays loads already prefetched 2 chunks ahead (ACT would
            # stall PSUM drains, gpsimd is not a hwdge engine)
            xp_eng = nc.sync
            for name in ("q", "k"):
                # contiguous transpose dst (fast path) + 32 slack cols so
                # the FWL-width lhsT slice of the last group stays in-bounds
                tt_ = xpT.tile([128, 32 * C2 + 32], F16, tag=f"{name}T",
                               name=f"{name}T{ci}")
                xp_eng.dma_start_transpose(
                    tt_[:, 0:32 * C2].rearrange("p (g c) -> p g c", c=C2),
                    dws[name])
                xT[(name, ci)] = tt_
            if ci >= 1:
                emit_attn(ci - 1)
                load_vp(ci - 1)
        load_vp(2)
        # first v group: ALL 9 taps on PE — keeps PE fed while chunk 3's
        # DVE tail + transpose land, and keeps the DVE FIFO clear so the
        # post-attn chain starts the moment attn3 completes.
        emit_vconv(0, taps=tuple(range(9)))
        emit_attn(NCHUNK - 1)

        # total sumsq -> [C2,1]
        nq2 = pool.tile([C2, 1], F32, tag="nq2")
        nk2 = pool.tile([C2, 1], F32, tag="nk2")
        for name, tgt in (("q", nq2), ("k", nk2)):
            terms = [sumsq[(name, ci, hh)] for ci in range(NCHUNK) for hh in range(2)]
            nc.vector.tensor_tensor(tgt[:], terms[0][:], terms[1][:], AOT.add)
            for t_ in terms[2:]:
                nc.vector.tensor_tensor(tgt[:], tgt[:], t_[:], AOT.add)

        # ---- post-attention (small) ----
        # A1[r, d] = attn[r, head(r)*48 + d]; head0 block straight from psum,
        # head1 block via an SBUF bounce (DMA cannot read PSUM, ACT cannot
        # start at partition 48)
        A1 = pool.tile([C2, CH], F32, tag="A1")
        A0 = pool.tile([C2, C2], F32, tag="A0")
        nc.scalar.copy(A1[0:CH, :], ps_attn[0:CH, 0:CH])
        nc.scalar.copy(A0[:], ps_attn[0:C2, :])
        nc.sync.dma_start(A1[CH:C2, :], A0[CH:C2, CH:C2])
        # rq = 1/sqrt(nq2), rk = 1/sqrt(nk2)
        rq = pool.tile([C2, 1], F32, tag="rq")
        rk = pool.tile([C2, 1], F32, tag="rk")
        for src2, dst in ((nq2, rq), (nk2, rk)):
            nc.scalar.sqrt(dst[:], src2[:])
            nc.vector.reciprocal(dst[:], dst[:])
        # column scaling by rk: build B [C2, CH]: rows 0:48 = rk[0:48]^T, rows 48:96 = rk[48:96]^T
        rk_dram = dram.tile([C2, 1], F32)
        nc.sync.dma_start(rk_dram[:], rk[:])
        Bc = pool.tile([C2, CH], F32, tag="Bc")
        rkd = rk_dram[:].rearrange("p one -> (p one)")
        nc.sync.dma_start(
            Bc[0:CH, :],
            rkd[0:CH].rearrange("(x e) -> x e", x=1).broadcast_to([CH, CH]))
        nc.sync.dma_start(
            Bc[CH:C2, :],
            rkd[CH:C2].rearrange("(x e) -> x e", x=1).broadcast_to([CH, CH]))
        nc.vector.tensor_tensor(A1[:], A1[:], Bc[:], AOT.mult)
        # rank count: G[r, d, e] = A1[r, e] > A1[r, d]  (free dims d,e),
        # processed in d-halves to halve the scratch footprint
        cnt = pool.tile([C2, CH], F32, tag="cnt")
        CH2 = CH // 2
        for dh in range(2):
            G = pool.tile([C2, CH2, CH], F16, tag="G", name=f"G{dh}")
            in_e = A1[:].rearrange("p (x e) -> p x e", x=1).broadcast_to([C2, CH2, CH])
            in_d = A1[:, dh * CH2:(dh + 1) * CH2].rearrange(
                "p (d x) -> p d x", x=1).broadcast_to([C2, CH2, CH])
            nc.vector.tensor_tensor(G[:], in_e, in_d, AOT.is_gt)
            nc.vector.tensor_reduce(cnt[:, dh * CH2:(dh + 1) * CH2], G[:],
                                    axis=mybir.AxisListType.X, op=AOT.add)
        # row scale = temp * rq; exp
        rsc = pool.tile([C2, 1], F32, tag="rsc")
        nc.vector.tensor_tensor(rsc[:], rq[:], tmpv[:], AOT.mult)
        E = pool.tile([C2, CH], F32, tag="E")
        nc.scalar.activation(E[:], A1[:], ACTF.Exp, scale=rsc[:])
        # 4 masked softmaxes combined, vectorized over the branch axis i:
        #   numer4[p,i,d] = E[p,d] * (cnt[p,d] < ks[i])
        #   Acc[p,d] = sum_i numer4[p,i,d] * attw[p,i] / den4[p,i]
        numer4 = pool.tile([C2, 4, CH], F32, tag="numer4")
        cnt_b = cnt[:].rearrange("p (x d) -> p x d", x=1).broadcast_to([C2, 4, CH])
        thr_b = ksthr_t[:].rearrange("p (i x) -> p i x", x=1).broadcast_to([C2, 4, CH])
        m4 = pool.tile([C2, 4, CH], F16, tag="m4")
        nc.vector.tensor_tensor(m4[:], cnt_b, thr_b, AOT.is_lt)
        E_b = E[:].rearrange("p (x d) -> p x d", x=1).broadcast_to([C2, 4, CH])
        nc.vector.tensor_tensor(numer4[:], E_b, m4[:], AOT.mult)
        den4 = pool.tile([C2, 4], F32, tag="den4")
        nc.vector.tensor_reduce(den4[:], numer4[:], axis=mybir.AxisListType.X,
                                op=AOT.add)
        rw4 = pool.tile([C2, 4], F32, tag="rw4")
        nc.vector.reciprocal(rw4[:], den4[:])
        nc.vector.tensor_tensor(rw4[:], rw4[:], attw[:], AOT.mult)
        rw4_b = rw4[:].rearrange("p (i x) -> p i x", x=1).broadcast_to([C2, 4, CH])
        nc.vector.tensor_tensor(numer4[:], numer4[:], rw4_b, AOT.mult)
        Acc = pool.tile([C2, CH], F32, tag="Acc")
        nc.vector.tensor_reduce(Acc[:], numer4[:].rearrange("p i d -> p d i"),
                                axis=mybir.AxisListType.X, op=AOT.add)
        # per-head A_comb f16 tiles (base_partition 0 for lhsT)
        Ah0 = pool.tile([CH, CH], F16, tag="Ah0")
        nc.vector.tensor_copy(Ah0[:], Acc[0:CH, :])
        Ah1 = pool.tile([CH, CH], F16, tag="Ah1")
        nc.gpsimd.dma_start(Ah1[:], Acc[CH:C2, :])
        Ah = [Ah0, Ah1]

        # ---- AllGather the tiny A_comb blocks (fp16, 9.2KB in) within
        # 4-core groups; the Wproj multiply happens post-gather on every
        # core (8 small matmuls) — far cheaper than shipping M_combT.
        b_in = dram.tile([C2, CH], F16)
        b_out = dram.tile([4, C2, CH], F16)
        nc.sync.dma_start(b_in[0:CH, :], Ah[0][:])
        nc.sync.dma_start(b_in[CH:C2, :], Ah[1][:])

        # v group 1 PE taps cover the collective trigger latency
        emit_vconv(1)

        nc.gpsimd.collective_compute(
            "AllGather", AOT.bypass,
            replica_groups=[[0, 1, 2, 3], [4, 5, 6, 7]],
            ins=[b_in.opt()], outs=[b_out.opt()])

        emit_vconv(2)

        # ---- M_combT: per head h, MT rows 48h..48h+48 = Ah_all[h].T @ wp_h
        AhAll = pool.tile([CH, HEADS, CH], F16, tag="AhAll")
        nc.sync.dma_start(
            AhAll[:], b_out[:].rearrange("g (h r) d -> r (g h) d", h=2))
        MT = pool.tile([128, 3, C], F16, tag="MT")
        for h in range(HEADS):
            ps = psum_o.tile([CH, C], F32, tag="psout", name=f"psmc{h}")
            nc.tensor.matmul(ps[:], AhAll[:, h, :], wpa[:, h, :],
                             start=True, stop=True)
            mt_h = obuf.tile([CH, C], F16, tag="mth", name=f"mth{h}")
            if h % 2 == 0:
                nc.scalar.copy(mt_h[:], ps[:])
            else:
                nc.vector.tensor_copy(mt_h[:], ps[:])
            r0 = CH * h
            while r0 < CH * (h + 1):
                kc, po = r0 // 128, r0 % 128
                nr = min(CH * (h + 1) - r0, 128 - po)
                nc.sync.dma_start(MT[po:po + nr, kc, :],
                                  mt_h[r0 - CH * h:r0 - CH * h + nr, :])
                r0 += nr

        # ---- final matmul: out[o, px] = sum_c MT[c, o] * vdw[c, px] ----
        for m in range(3):
            for n in range(VPX // SUB):
                ps = psum_o.tile([128, SUB], F32, tag="psout")
                for kc in range(3):
                    nc.tensor.matmul(
                        ps[:], MT[:, kc, 128 * m:128 * (m + 1)],
                        vdw[:, kc, SUB * n:SUB * (n + 1)],
                        start=(kc == 0), stop=(kc == 2))
                ob = obuf.tile([128, SUB], F16, tag="ob")
                if n % 2 == 0:
                    nc.scalar.copy(ob[:], ps[:])
                else:
                    nc.vector.tensor_copy(ob[:], ps[:])
                nc.sync.dma_start(out_ext.ap()[m, :, SUB * n:SUB * (n + 1)], ob[:])

    if CFG["split_waits"]:
        _split_multi_waits(nc, CFG["max_waits"])
    return nc


# ---------------- host-side sharding ----------------

def _prep_inputs(q_fea, k_fea, v_fea, wq, wk, wv, wproj, temperature, attn_w):
    q_fea = np.asarray(q_fea, np.float32)
    k_fea = np.asarray(k_fea, np.float32)
    v_fea = np.asarray(v_fea, np.float32)
    wq = np.asarray(wq, np.float32)[:, 0]      # [C,3,3]
    wk = np.asarray(wk, np.float32)[:, 0]
    wv = np.asarray(wv, np.float32)[:, 0]
    wproj = np.asarray(wproj, np.float32)[:, :, 0, 0]  # [C,C]
    temp = np.asarray(temperature, np.float32).reshape(HEADS)
    attn_w = np.asarray(attn_w, np.float32).reshape(4)

    wq9 = wq.reshape(C, 9)
    wk9 = wk.reshape(C, 9)
    wv9 = wv.reshape(C, 9)

    dgv = np.zeros((9, 3, 128, 128), np.float16)
    for t in range(9):
        for ct in range(3):
            np.fill_diagonal(dgv[t, ct], wv9[128 * ct:128 * (ct + 1), t].astype(np.float16))

    in_maps = []
    for core in range(8):
        b = core // 4
        g = core % 4
        ch0 = C2 * g
        r0 = VROWS * g

        def padqk(x):
            p = np.zeros((C2, 130, 130), np.float16)
            p[:, 1:129, 1:129] = x[b, ch0:ch0 + C2]
            return p

        vp = np.zeros((C, 34, 130), np.float16)
        glo = max(0, r0 - 1)
        ghi = min(H, r0 + VROWS + 1)
        vp[:, glo - (r0 - 1):ghi - (r0 - 1), 1:129] = v_fea[b, :, glo:ghi, :]

        dgqk = np.zeros((2, 9, C2, 128), np.float16)
        for t in range(9):
            np.fill_diagonal(dgqk[0, t, :, 0:C2], wq9[ch0:ch0 + C2, t].astype(np.float16))
            np.fill_diagonal(dgqk[1, t, :, 0:C2], wk9[ch0:ch0 + C2, t].astype(np.float16))

        in_maps.append({
            "qs": padqk(q_fea),
            "ks": padqk(k_fea),
            "vs": vp,
            "wq9": np.ascontiguousarray(wq9[ch0:ch0 + C2]),
            "wk9": np.ascontiguousarray(wk9[ch0:ch0 + C2]),
            "wv9": wv9,
            "dgqk": dgqk,
            "dgv": dgv,
            "wpT": np.ascontiguousarray(
                wproj.T.reshape(HEADS, CH, C).transpose(1, 0, 2)).astype(np.float16),
            "tempv": np.repeat(temp[2 * g + HEADS * 0:2 * g + 2], CH)[:, None].copy(),
            "attwv": np.tile(attn_w, (C2, 1)),
            "ksthr": np.tile(np.array(KS, np.float32), (C2, 1)),
        })
    return in_maps


def _assemble(results):
    out = np.zeros((B, C, H, W), np.float32)
    for core in range(8):
        b = core // 4
        r0 = VROWS * (core % 4)
        o = results[core]["out"].astype(np.float32)  # [3, 128, VPX]
        out[b, :, r0:r0 + VROWS, :] = o.reshape(C, VROWS, W)
    return out


_CACHE = {}


def kernel(**inputs) -> np.ndarray:
    if "nc" not in _CACHE:
        _CACHE["nc"] = build_kernel()
    nc = _CACHE["nc"]
    in_maps = _prep_inputs(**inputs)
    res = run_bass_kernel_spmd(nc, in_maps, core_ids=list(range(8)))
    return _assemble(res.results)


if __name__ == "__main__":
    sys.path.insert(0, "/root/problem")
    from reference import setup_inputs, reference

    inputs = setup_inputs()
    ref = np.asarray(reference(**inputs))
    got = kernel(**{k: np.asarray(v) for k, v in inputs.items()})
    rel = np.linalg.norm(got - ref) / np.linalg.norm(ref)
    print(f"Relative error: {rel:.3e}")

